# revision 51
# baseline (speedup 1.0000x reference)
"""ChildSum TreeLSTM (N=8192 nodes, 4-ary static heap tree, H=256, D=300) on 8 trn2 NeuronCores.

Strategy
--------
The tree is static: node i's children are 4i+1..4i+4 (clipped at N). The reverse
scan (children before parents) is equivalent to processing the tree level by
level, bottom-up; nodes within a level are independent, so each level is a
batched LSTM cell (matmuls + elementwise).

Sharding: the 256 level-4 subtrees are partitioned across the 8 cores (balanced
by the number of *internal* level-6 descendants, which determines level-7 leaf
count). Each core processes its forest fully locally — children of a sorted node
range are contiguous in the next level's sorted array, so the recurrence needs
no gathers and no cross-core communication. Cores output their 32 level-4 root
(h, c) states; the tiny top of the tree (levels 3..0, 85 nodes) plus the final
log_softmax run on the host in numpy.

On-device layout: everything is transposed — feature dim on SBUF partitions
(256 features = 2 halves of 128), nodes along the free axis. The child-h sums
and per-child forget gates then become strided slicing along the free axis.
Key perf choices (round 2):
  * bf16 weights + h-state: avoids the f32r 4x small-matmul penalty on the
    recurrence matmuls, halves weight DMA, and unlocks DVE 2x/4x modes.
  * inputs repacked on host to [128, k, cols] so each (queue, priority) chunk
    is ONE large DMA; 9 input DMAs total spread over 4 queues, ordered so the
    first range's operands land first.
  * x-side projections in three uniform 352-column ranges; the forget gate is
    only computed for the 256 internal-node columns (leaves don't use it).
  * leaf-column gate activations are applied directly while draining PSUM
    (act-drain) — raw gx never round-trips through SBUF for leaf columns.
  * biases (bx + bh, zeros in practice) are folded into an extra ones-row of
    the x-side matmul, so pad columns (zero x) self-compute to h = c = 0.
"""

import numpy as np
import ml_dtypes

BF16 = ml_dtypes.bfloat16
FP8 = ml_dtypes.float8_e4m3fn

N = 8192
H = 256
D = 300
K = 4
OUT = 4
NCORES = 8
L7P = 384           # padded level-7 columns per core (4 * IPMAX)
IPMAX = 96          # max internal level-6 nodes per core
KUSE = 301          # contraction rows actually used (300 emb + 1 ones)
XCOLS = L7P + 512 + 128   # 1024 per-core node columns: [L7 | L6 | L5]
XS = 32.0           # fp8 quantization scale for embeddings
WS = 8.0            # fp8 quantization scale for Wx
DESCALE = 1.0 / (XS * WS)

GATE_MAP = [0, 2, 3, 1]  # our gate order [i, o, u, f] -> reference gate indices

F32 = np.float32


def _build_plan():
    """Assign the 256 level-4 subtrees to 8 cores; build per-core column maps."""
    # w(u) = number of internal (has-children) level-6 descendants of L4 node u.
    # Full-weight subtrees (w=16) are u in [85, 127); u=127 has w=11; rest 0.
    full = list(range(85, 127))                               # 42 subtrees
    lights = list(range(128, 341))                            # 213 subtrees
    heavy_counts = [6, 6, 5, 5, 5, 5, 5, 5]                   # sums to 42
    light_counts = [26, 26, 26, 27, 27, 27, 27, 27]           # sums to 213
    cores = []
    hpos = 0
    lpos = 0
    for c in range(NCORES):
        hs = full[hpos:hpos + heavy_counts[c]]
        hpos += heavy_counts[c]
        if c == 2:
            hs = hs + [127]                                   # w sums: 96,96,91,80*5
        ls = lights[lpos:lpos + light_counts[c]]
        lpos += light_counts[c]
        cores.append(sorted(hs + ls))
    all_l4 = sorted(u for cs in cores for u in cs)
    assert all_l4 == list(range(85, 341)), "L4 assignment must partition [85, 341)"

    plan = []
    for c in range(NCORES):
        l4 = cores[c]
        assert len(l4) == 32
        l5 = [4 * u + 1 + k for u in l4 for k in range(K)]
        l6 = [4 * v + 1 + k for v in l5 for k in range(K)]
        wc = sum(1 for x in l6 if x < 2048)
        assert wc <= IPMAX
        l7 = []
        for x in l6[:wc]:
            for k in range(K):
                ch = 4 * x + 1 + k
                l7.append(ch if ch < N else -1)
        l7 += [-1] * (L7P - len(l7))
        cols = np.array(l7 + l6 + l5, dtype=np.int64)
        assert cols.shape == (XCOLS,)
        plan.append((cols, wc, np.array(l5, dtype=np.int64)))
    return plan


_PLAN = _build_plan()

# chunk schedule: (xoff, ncols, ip, child_level, child_col_off, out_level,
# out_off, goff).  goff = column offset of this chunk's internal nodes inside
# the 224-wide internal-gx tiles (GXI / GX3): [L6int 0:96 | L5 96:224].
# Level 4 and above run on the host.  The DAG is split so the bulk of L5
# (nodes 24:128, whose children are all leaf-L6) depends only on the leaf
# chunks and runs concurrently with the L6-internal -> L5-head chain:
#   c1,c2 (L7) -> c4 (L6 int, SH6[0:96]) -> c6 (L5 head, SH5[0:24])
#   c3 (L6 leaf, SH6[96:512]) -> c5 (L5 bulk, SH5[24:128])
# last field: elementwise engine for the chunk's chain ('v' vector /
# 'g' gpsimd) - two concurrent chains must not share one engine queue.
_CHUNKS = [
    (0,    256, 0,   None, 0,   7, 0,   None, 'v'),  # c1: L7 leaves, part A
    (256,  128, 0,   None, 0,   7, 256, None, 'v'),  # c2: L7 leaves, part B
    (480,  416, 0,   None, 0,   6, 96,  None, 'v'),  # c3: L6 leaf cols
    (920,  104, 104, 6,    96,  5, 24,  120,  'v'),  # c5: L5 bulk (leaf kids)
    (384,  96,  96,  7,    0,   6, 0,   0,    'v'),  # c4: L6 internal
]
_STATE_COLS = {7: L7P, 6: 512, 5: 128}

# phase-1 column ranges (each <= 512 PSUM f32 columns)
_RANGES = [(0, 352), (352, 704), (704, 1024)]
# per range: internal windows (copy-drained to GXI / matmul'd for the f
# gate), with dst offset in the 224-wide internal tiles.
_INT_WIN = [[], [(384, 480, 0)], [(896, 1024, 96)]]


def _static_tree():
    idx = np.arange(N)[:, None] * K + 1 + np.arange(K)[None, :]
    mask = (idx < N).astype(F32)
    idx = np.where(idx < N, idx, 0).astype(np.int32)
    return idx, mask


_STATIC_IDX, _STATIC_MASK = _static_tree()


def _pack_weights(Wx, bx, Wh, bh):
    """Pack to partition-major HBM layouts.

    wx3  [128, 3, 1024] fp8e4m3 (x WS): wx3[p, k, 256*g + j] = Wx[rg][j, 128k+p]
                               row 301 (k=2, p=45) holds (bx+bh) * WS.
    wh2  [128, 2, 768]  bf16 : i/o/u recurrence weights, transposed.
    whf2 [128, 2, 256]  bf16 : forget recurrence weights, transposed.
    """
    wx = np.zeros((384, 4 * H), dtype=F32)
    for g, rg in enumerate(GATE_MAP):
        wx[:D, H * g:H * (g + 1)] = np.asarray(Wx[rg], dtype=F32).T
        wx[D, H * g:H * (g + 1)] = np.asarray(bx[rg], dtype=F32) + np.asarray(bh[rg], dtype=F32)
    wx3 = np.ascontiguousarray(
        (wx * WS).reshape(3, 128, 4 * H).transpose(1, 0, 2)).astype(FP8)
    wh = np.zeros((H, 3 * H), dtype=F32)
    for g, rg in enumerate([0, 2, 3]):  # i, o, u
        wh[:, H * g:H * (g + 1)] = np.asarray(Wh[rg], dtype=F32).T
    wh2 = np.ascontiguousarray(wh.reshape(2, 128, 3 * H).transpose(1, 0, 2)).astype(BF16)
    whf = np.asarray(Wh[1], dtype=F32).T
    whf2 = np.ascontiguousarray(whf.reshape(2, 128, H).transpose(1, 0, 2)).astype(BF16)
    return wx3, wh2, whf2


def _pack_xt(xs, emb_table):
    """Per-core transposed embeddings, partition-major: [128, 3, XCOLS] fp8."""
    X = np.asarray(emb_table, dtype=F32)[np.asarray(xs)]
    xts = []
    for cols, _, _ in _PLAN:
        xt = np.zeros((384, XCOLS), dtype=F32)
        real = cols >= 0
        xt[:D, real] = X[cols[real]].T
        xt[D, real] = 1.0
        xt3 = np.ascontiguousarray(
            (xt * XS).reshape(3, 128, XCOLS).transpose(1, 0, 2)).astype(FP8)
        xts.append(xt3)
    return xts


def _sigmoid(x):
    return (1.0 / (1.0 + np.exp(-x))).astype(F32)


def _host_top(Hbuf, Cbuf, xs, emb_table, Wx, bx, Wh, bh):
    """Compute tree levels 4..0 (nodes 0..340) on the host, numpy fp32."""
    Wx = np.asarray(Wx, dtype=F32)
    bx = np.asarray(bx, dtype=F32)
    Wh = np.asarray(Wh, dtype=F32)
    bh = np.asarray(bh, dtype=F32)
    emb = np.asarray(emb_table, dtype=F32)
    xs = np.asarray(xs)
    for lo, hi in [(85, 341), (21, 85), (5, 21), (1, 5), (0, 1)]:
        ids = np.arange(lo, hi)
        Xl = emb[xs[ids]]                                   # [n, D]
        gx = np.einsum('ghd,nd->ngh', Wx, Xl).astype(F32) + bx
        cidx = ids[:, None] * K + 1 + np.arange(K)[None, :]  # all valid (< 341)
        Hc = Hbuf[cidx]
        Cc = Cbuf[cidx]
        hs = Hc.sum(1)
        ig = _sigmoid(gx[:, 0] + hs @ Wh[0].T + bh[0])
        og = _sigmoid(gx[:, 2] + hs @ Wh[2].T + bh[2])
        ug = np.tanh(gx[:, 3] + hs @ Wh[3].T + bh[3]).astype(F32)
        f = _sigmoid(gx[:, 1][:, None, :] + Hc @ Wh[1].T + bh[1])
        cc = ig * ug + (f * Cc).sum(1)
        hh = og * np.tanh(cc).astype(F32)
        Hbuf[ids] = hh
        Cbuf[ids] = cc
    return Hbuf[0]


def _log_softmax(x):
    m = np.max(x)
    e = np.exp(x - m)
    return (x - m - np.log(e.sum())).astype(F32)


# ----------------------------------------------------------------------------
# Bass device program
# ----------------------------------------------------------------------------

_COMPILED = None


def _build_device_program():
    import contextlib

    import concourse.bacc as bacc
    import concourse.tile as tile
    import concourse.mybir as mybir

    f32 = mybir.dt.float32
    bf16 = mybir.dt.bfloat16
    fp8 = mybir.dt.float8e4
    Sig = mybir.ActivationFunctionType.Sigmoid
    Tanh = mybir.ActivationFunctionType.Tanh
    AxX = mybir.AxisListType.X
    Add = mybir.AluOpType.add
    DRow = mybir.MatmulPerfMode.DoubleRow

    nc = bacc.Bacc("TRN2", target_bir_lowering=False, debug=False,
                   num_devices=NCORES)

    xt_d = nc.dram_tensor("xt", [128, 3, XCOLS], fp8, kind="ExternalInput")
    wx_d = nc.dram_tensor("wx", [128, 3, 4 * H], fp8, kind="ExternalInput")
    wh_d = nc.dram_tensor("wh", [128, 2, 3 * H], bf16, kind="ExternalInput")
    whf_d = nc.dram_tensor("whf", [128, 2, H], bf16, kind="ExternalInput")
    eye_d = nc.dram_tensor("eye", [128, 128], bf16, kind="ExternalInput")
    out_h_d = nc.dram_tensor("out_h", [128, 2, 104], bf16, kind="ExternalOutput")
    out_c_d = nc.dram_tensor("out_c", [128, 2, 104], bf16, kind="ExternalOutput")
    out_h6_d = nc.dram_tensor("out_h6", [128, 2, 96], bf16, kind="ExternalOutput")
    out_c6_d = nc.dram_tensor("out_c6", [128, 2, 96], bf16, kind="ExternalOutput")

    GFUNC = [Sig, Sig, Tanh]   # activation per gate i, o, u

    with tile.TileContext(nc) as tc:
        with contextlib.ExitStack() as ctx:
            inp = ctx.enter_context(tc.tile_pool(name="inp", bufs=1))
            st = ctx.enter_context(tc.tile_pool(name="state", bufs=1))
            wk = ctx.enter_context(tc.tile_pool(name="work", bufs=2))

            # --- input SBUF tiles (single tile per tensor; sliced DMAs)
            xt_s = inp.tile([128, 3, XCOLS], fp8, tag="xt", name="xt")
            wx_s = inp.tile([128, 3, 4 * H], fp8, tag="wx", name="wx")
            wh_s = inp.tile([128, 2, 3 * H], bf16, tag="wh", name="wh")
            whf_s = inp.tile([128, 2, H], bf16, tag="whf", name="whf")
            eye_s = inp.tile([128, 128], bf16, tag="eye", name="eye")

            # priority DMAs. Only sync / scalar / gpsimd queues can issue
            # DMAs; scalar is kept DMA-free so its ACT_TABLE_LOADs and
            # act-drains are never queued behind DMA issues.
            a0, b0 = _RANGES[0]
            nc.gpsimd.dma_start(out=wx_s[:, :, 512:768], in_=wx_d[:, :, 512:768])
            nc.sync.dma_start(out=xt_s[:, 0, a0:b0], in_=xt_d[:, 0, a0:b0])
            nc.sync.dma_start(out=xt_s[:, 1, a0:b0], in_=xt_d[:, 1, a0:b0])
            nc.gpsimd.dma_start(out=xt_s[:, 2, a0:b0], in_=xt_d[:, 2, a0:b0])
            nc.gpsimd.dma_start(out=eye_s[:], in_=eye_d[:])
            # remaining ranges / quarters, less urgent
            nc.sync.dma_start(out=xt_s[:, :, _RANGES[1][0]:_RANGES[1][1]],
                              in_=xt_d[:, :, _RANGES[1][0]:_RANGES[1][1]])
            nc.gpsimd.dma_start(out=wx_s[:, :, 0:256], in_=wx_d[:, :, 0:256])
            nc.sync.dma_start(out=xt_s[:, :, _RANGES[2][0]:_RANGES[2][1]],
                              in_=xt_d[:, :, _RANGES[2][0]:_RANGES[2][1]])
            nc.gpsimd.dma_start(out=wx_s[:, :, 256:512], in_=wx_d[:, :, 256:512])
            nc.gpsimd.dma_start(out=wx_s[:, :, 768:1024],
                                in_=wx_d[:, :, 768:1024])
            nc.sync.dma_start(out=wh_s[:], in_=wh_d[:])
            nc.sync.dma_start(out=whf_s[:], in_=whf_d[:])

            # --- persistent state + gate tiles
            SH = {lv: st.tile([128, 2, n], bf16, tag=f"h{lv}", name=f"sh{lv}")
                  for lv, n in _STATE_COLS.items()}
            SC = {lv: st.tile([128, 2, n], bf16, tag=f"c{lv}", name=f"sc{lv}")
                  for lv, n in _STATE_COLS.items()}
            # activated gates for ALL columns (leaf cols filled by act-drain
            # in phase 1; internal cols filled during phase 2)
            G = [st.tile([128, 2, XCOLS], bf16, tag=f"g{g}", name=f"g{g}")
                 for g in range(3)]
            # raw gx for internal columns only (i/o/u for the psum add; f for
            # the per-child forget bias)
            # raw (fp8-scaled) gx, bf16: consumed via identity-matmul psum
            # preloads which fold in the descale
            GXI = [st.tile([128, 2, 224], bf16, tag=f"gxi{g}", name=f"gxi{g}")
                   for g in range(3)]
            GX3 = st.tile([128, 2, 224], bf16, tag="gx3", name="gx3")

            # --- PE warm-up: ramp the tensor engine to full clock during
            # the input-DMA window (the p-state ramp needs ~3us of
            # continuous execution; these matmuls depend only on a memset).
            wz = wk.tile([128, 512], bf16, tag="wz", name="wz")
            nc.vector.memset(wz[:], 0.0)
            with tc.tile_pool(name="psum0", bufs=1, space="PSUM") as ps0:
                for w in range(10):
                    pw = ps0.tile([128, 512], f32, tag="pw", bufs=2,
                                  name=f"pw{w}")
                    nc.tensor.matmul(pw[:], wz[:, 0:128], wz[:],
                                     start=True, stop=True)

            with nc.allow_low_precision("bf16 gates/h-state within 2e-2 tol"):
                # --- phase 1: x-side projections, range-major sweep.
                # leaf windows act-drained straight into G; internal windows
                # copy-drained into GXI; f-gate computed only on internal
                # windows into a single psum tile, drained to GX3.
                # Phase-1 PSUM lives in its own scoped pool so its banks are
                # released to the phase-2 pool (pa/pf) afterwards.
                ps1_cm = tc.tile_pool(name="psum1", bufs=1, space="PSUM")
                ps = ps1_cm.__enter__()
                gxf = ps.tile([128, 2, 224], f32, tag="gxf", name="gxf")
                for ri, (a, b) in enumerate(_RANGES):
                    for m in (4, 5, 0, 1, 2, 3):   # u first: c = i*u unblocks
                        g, phi = divmod(m, 2)
                        col = 128 * m
                        pt = ps.tile([128, 512], f32, tag="gx", bufs=3,
                                     name=f"pgx{ri}_{m}")
                        # fp8 DoubleRow: k-subtiles 0+1 in one half-rate pass,
                        # the 45-row remainder (k=2, incl. bias row) single.
                        nc.tensor.matmul(
                            pt[:, 0:b - a],
                            wx_s[:, 0:2, col:col + 128],
                            xt_s[:, 0:2, a:b],
                            start=True, stop=False, perf_mode=DRow)
                        nc.tensor.matmul(
                            pt[:, 0:b - a],
                            wx_s[:, 2, col:col + 128],
                            xt_s[:, 2, a:b],
                            start=False, stop=True)
                        # act-drain the whole range into G with the fp8
                        # descale folded into the activation's input scale
                        # (internal cols are overwritten by phase 2); raw gx
                        # for internal cols is copy-drained separately.
                        nc.scalar.activation(
                            G[g][:, phi, a:b], pt[:, 0:b - a], GFUNC[g],
                            scale=DESCALE)
                        for (wa, wb, go) in _INT_WIN[ri]:
                            nc.vector.tensor_copy(
                                GXI[g][:, phi, go:go + wb - wa],
                                pt[:, wa - a:wb - a])
                    for (wa, wb, go) in _INT_WIN[ri]:
                        for phi in range(2):
                            col = 768 + 128 * phi
                            nc.tensor.matmul(
                                gxf[:, phi, go:go + wb - wa],
                                wx_s[:, 0:2, col:col + 128],
                                xt_s[:, 0:2, wa:wb],
                                start=True, stop=False, perf_mode=DRow)
                            nc.tensor.matmul(
                                gxf[:, phi, go:go + wb - wa],
                                wx_s[:, 2, col:col + 128],
                                xt_s[:, 2, wa:wb],
                                start=False, stop=True)
                nc.vector.tensor_copy(GX3[:], gxf[:])
                ps1_cm.__exit__(None, None, None)
                ps2_cm = tc.tile_pool(name="psum2", bufs=1, space="PSUM")
                ps = ps2_cm.__enter__()

                # --- phase 2: levels bottom-up in chunks
                for (xoff, cn, ip, child, coff, outlv, ooff, goff,
                     ech) in _CHUNKS:
                    if ip > 0:
                        # hs = sum of the 4 child h columns per node
                        hs = wk.tile([128, 2, ip], bf16, tag="hs", name="hs")
                        cv = SH[child][:, :, coff:coff + 4 * ip].rearrange(
                            "p t (n k) -> p t n k", k=K)
                        nc.vector.tensor_reduce(hs[:], cv, axis=AxX, op=Add)

                        # i/o/u gates: preload gx through the PE (identity
                        # matmul with the fp8 descale folded into eye), then
                        # accumulate wh @ hs on top.
                        Ps = []
                        for g in range(3):
                            P = ps.tile([128, 2, 128], f32, tag="pa", bufs=3,
                                        name=f"pa{outlv}_{g}")
                            nc.tensor.matmul(
                                P[:, :, 0:ip], eye_s[:],
                                GXI[g][:, :, goff:goff + ip],
                                start=True, stop=False,
                                skip_group_check=True)
                            Ps.append(P)
                        Pfc = ps.tile([128, 2, 512], f32, tag="pf", bufs=2,
                                      name="pfc")
                        for phi in range(2):
                            gfb = GX3[:, phi, goff:goff + ip][:, :, None]
                            nc.tensor.matmul(
                                Pfc[:, phi, 0:4 * ip], eye_s[:],
                                gfb.broadcast_to([128, ip, K]),
                                start=True, stop=False,
                                skip_group_check=True)

                        for g in range(3):
                            P = Ps[g]
                            for phi in range(2):
                                for k in range(2):
                                    nc.tensor.matmul(
                                        P[:, phi, 0:ip],
                                        wh_s[:, k, 256 * g + 128 * phi:
                                             256 * g + 128 * phi + 128],
                                        hs[:, k, 0:ip],
                                        start=False, stop=(k == 1),
                                        skip_group_check=True)
                            nc.scalar.activation(
                                G[g][:, :, xoff:xoff + ip], P[:, :, 0:ip],
                                GFUNC[g])

                        # f = sigmoid(gf + Whf @ h_child) per child;
                        # csum = sum_k f_k * c_child_k
                        FS = wk.tile([128, 2, 512], bf16, tag="fs", name="fs")
                        for phi in range(2):
                            for k in range(2):
                                nc.tensor.matmul(
                                    Pfc[:, phi, 0:4 * ip],
                                    whf_s[:, k, 128 * phi:128 * phi + 128],
                                    SH[child][:, k, coff:coff + 4 * ip],
                                    start=False, stop=(k == 1),
                                    skip_group_check=True)
                            nc.scalar.activation(
                                FS[:, phi, 0:4 * ip], Pfc[:, phi, 0:4 * ip],
                                Sig)
                        nc.vector.tensor_mul(
                            FS[:, :, 0:4 * ip], FS[:, :, 0:4 * ip],
                            SC[child][:, :, coff:coff + 4 * ip])
                        sv = FS[:, :, 0:4 * ip].rearrange(
                            "p t (n k) -> p t n k", k=K)
                        csum = wk.tile([128, 2, ip], bf16, tag="csum",
                                       name="csum")
                        nc.vector.tensor_reduce(csum[:], sv, axis=AxX, op=Add)

                    # c = ig*ug (+ csum on internal cols); h = og*tanh(c)
                    Cdst = SC[outlv][:, :, ooff:ooff + cn]
                    nc.vector.tensor_mul(
                        Cdst, G[0][:, :, xoff:xoff + cn],
                        G[2][:, :, xoff:xoff + cn])
                    if ip > 0:
                        nc.vector.tensor_add(
                            SC[outlv][:, :, ooff:ooff + ip],
                            SC[outlv][:, :, ooff:ooff + ip],
                            csum[:])
                    TC = wk.tile([128, 2, 512], bf16, tag="tc", name="tc")
                    nc.scalar.activation(TC[:, :, 0:cn], Cdst, Tanh)
                    nc.vector.tensor_mul(
                        SH[outlv][:, :, ooff:ooff + cn],
                        G[1][:, :, xoff:xoff + cn], TC[:, :, 0:cn])

            ps2_cm.__exit__(None, None, None)
            nc.sync.dma_start(out=out_c6_d[:], in_=SC[6][:, :, 0:96])
            nc.sync.dma_start(out=out_c_d[:], in_=SC[5][:, :, 24:128])
            nc.sync.dma_start(out=out_h6_d[:], in_=SH[6][:, :, 0:96])
            nc.sync.dma_start(out=out_h_d[:], in_=SH[5][:, :, 24:128])

    nc.compile()
    return nc


def _get_compiled():
    global _COMPILED
    if _COMPILED is None:
        _COMPILED = _build_device_program()
    return _COMPILED


def _numpy_fallback(xs, child_idx, child_mask, emb_table, Wx, bx, Wh, bh,
                    Wout, bout):
    """Exact sequential scan in numpy; only used if the tree is not the
    expected static 4-ary heap."""
    X = np.asarray(emb_table, dtype=F32)[np.asarray(xs)]
    Wx = np.asarray(Wx, dtype=F32)
    Wh = np.asarray(Wh, dtype=F32)
    bx = np.asarray(bx, dtype=F32)
    bh = np.asarray(bh, dtype=F32)
    gx = np.einsum('ghd,nd->ngh', Wx, X).astype(F32) + bx
    Hb = np.zeros((N, H), dtype=F32)
    Cb = np.zeros((N, H), dtype=F32)
    ci = np.asarray(child_idx)
    cm = np.asarray(child_mask, dtype=F32)
    for i in range(N - 1, -1, -1):
        idx = ci[i]
        m = cm[i][:, None]
        Hc = Hb[idx] * m
        Cc = Cb[idx] * m
        hs = Hc.sum(0)
        g = gx[i]
        ig = _sigmoid(g[0] + Wh[0] @ hs + bh[0])
        og = _sigmoid(g[2] + Wh[2] @ hs + bh[2])
        ug = np.tanh(g[3] + Wh[3] @ hs + bh[3]).astype(F32)
        f = _sigmoid(g[1] + Hc @ Wh[1].T + bh[1])
        c = ig * ug + (f * Cc).sum(0)
        Hb[i] = og * np.tanh(c).astype(F32)
        Cb[i] = c
    logits = np.asarray(Wout, dtype=F32) @ Hb[0] + np.asarray(bout, dtype=F32)
    return _log_softmax(logits)


def kernel(xs, child_idx, child_mask, emb_table, Wx, bx, Wh, bh, Wout, bout):
    xs = np.asarray(xs)
    if not (np.array_equal(np.asarray(child_idx), _STATIC_IDX)
            and np.array_equal(np.asarray(child_mask, dtype=F32), _STATIC_MASK)):
        return _numpy_fallback(xs, child_idx, child_mask, emb_table, Wx, bx,
                               Wh, bh, Wout, bout)

    from concourse.bass_utils import run_bass_kernel_spmd

    wx3, wh2, whf2 = _pack_weights(Wx, bx, Wh, bh)
    xts = _pack_xt(xs, emb_table)
    eye = np.ascontiguousarray(np.eye(128, dtype=F32) * DESCALE).astype(BF16)
    in_maps = [
        {"xt": xts[c], "wx": wx3, "wh": wh2, "whf": whf2, "eye": eye}
        for c in range(NCORES)
    ]
    nc = _get_compiled()
    res = run_bass_kernel_spmd(nc, in_maps, core_ids=list(range(NCORES)))

    def _unpack(a):  # [128, 2, n] feature-major halves -> [n, 256]
        a = np.asarray(a, dtype=F32)
        return np.concatenate([a[:, 0, :], a[:, 1, :]], axis=0).T

    Hbuf = np.zeros((1365, H), dtype=F32)
    Cbuf = np.zeros((1365, H), dtype=F32)
    l5h_ids = []     # L5-head nodes (computed on host from L6-head states)
    H6 = []
    C6 = []
    for c in range(NCORES):
        cols, _, l5 = _PLAN[c]
        Hbuf[l5[24:128]] = _unpack(res.results[c]["out_h"])
        Cbuf[l5[24:128]] = _unpack(res.results[c]["out_c"])
        l5h_ids.append(l5[0:24])
        H6.append(_unpack(res.results[c]["out_h6"]))   # [96, 256]
        C6.append(_unpack(res.results[c]["out_c6"]))
    ids = np.concatenate(l5h_ids)                       # [192]
    Hc = np.concatenate(H6).reshape(-1, K, H)           # [192, 4, 256]
    Cc = np.concatenate(C6).reshape(-1, K, H)
    WxF = np.asarray(Wx, dtype=F32)
    WhF = np.asarray(Wh, dtype=F32)
    bxF = np.asarray(bx, dtype=F32)
    bhF = np.asarray(bh, dtype=F32)
    Xl = np.asarray(emb_table, dtype=F32)[xs[ids]]
    gx = np.einsum('ghd,nd->ngh', WxF, Xl).astype(F32) + bxF
    hsum = Hc.sum(1)
    ig = _sigmoid(gx[:, 0] + hsum @ WhF[0].T + bhF[0])
    og = _sigmoid(gx[:, 2] + hsum @ WhF[2].T + bhF[2])
    ug = np.tanh(gx[:, 3] + hsum @ WhF[3].T + bhF[3]).astype(F32)
    f = _sigmoid(gx[:, 1][:, None, :] + Hc @ WhF[1].T + bhF[1])
    cc = ig * ug + (f * Cc).sum(1)
    Hbuf[ids] = og * np.tanh(cc).astype(F32)
    Cbuf[ids] = cc

    h0 = _host_top(Hbuf, Cbuf, xs, emb_table, Wx, bx, Wh, bh)
    logits = np.asarray(Wout, dtype=F32) @ h0 + np.asarray(bout, dtype=F32)
    return _log_softmax(logits)
